# revision 1
# baseline (speedup 1.0000x reference)
"""Trainium2 Bass kernel: GQA causal sliding-window self-attention.

Sharding: 8 cores = DP2 (batch) x TP4 (head groups). Core c: b=c//4, tp=c%4.
Each core: 4 q-heads, 2 kv-heads, wproj input-slice; host sums 4 TP partials.
"""
import sys

sys.path.insert(0, "/opt/trn_rl_repo")

import numpy as np
import ml_dtypes

import concourse.bass as bass
import concourse.mybir as mybir
import concourse.tile as tile
from concourse import bacc
from concourse.bass_utils import run_bass_kernel_spmd
from concourse.masks import make_identity

bf16 = ml_dtypes.bfloat16
FP32 = mybir.dt.float32
BF16 = mybir.dt.bfloat16
T = 2048
NT = 16          # t tiles of 128
NCC = 16         # contraction chunks of 128 over C=2048
EPS = float(np.finfo(np.float32).eps)
AF = mybir.ActivationFunctionType
ALU = mybir.AluOpType
AX = mybir.AxisListType

_CACHE = {}


def _bcast_mid(ap, n):
    """Insert a 0-stride dim of size n after the partition dim."""
    return bass.AP(ap.tensor, ap.offset, [list(ap.ap[0]), [0, n], *[list(d) for d in ap.ap[1:]]])


def _build_nc():
    nc = bacc.Bacc(None, target_bir_lowering=False)

    xT = nc.dram_tensor("xT", [2048, 2048], BF16, kind="ExternalInput")
    ve2 = nc.dram_tensor("ve2", [2048, 256], BF16, kind="ExternalInput")
    wqkv = nc.dram_tensor("wqkv", [2048, 1024], BF16, kind="ExternalInput")
    wp = nc.dram_tensor("wp", [512, 2048], BF16, kind="ExternalInput")
    wveg = nc.dram_tensor("wveg", [32, 2], BF16, kind="ExternalInput")
    wag = nc.dram_tensor("wag", [12, 4], BF16, kind="ExternalInput")
    cosb = nc.dram_tensor("cosb", [2048, 64], BF16, kind="ExternalInput")
    sinb = nc.dram_tensor("sinb", [2048, 64], BF16, kind="ExternalInput")
    mdiag = nc.dram_tensor("mdiag", [128, 128], BF16, kind="ExternalInput")
    mfar = nc.dram_tensor("mfar", [128, 128], BF16, kind="ExternalInput")
    out = nc.dram_tensor("out", [2048, 2048], FP32, kind="ExternalOutput")

    with tile.TileContext(nc) as tc:
        with (
            tc.tile_pool(name="big", bufs=1) as big,
            tc.tile_pool(name="work", bufs=2) as work,
            tc.tile_pool(name="small", bufs=4) as small,
        ):
            # ---- resident inputs ----
            xT_sb = big.tile([128, NCC, 2048], BF16)
            for cc in range(NCC):
                nc.sync.dma_start(out=xT_sb[:, cc, :], in_=xT[bass.ts(cc, 128), :])
            wqkv_sb = big.tile([128, NCC, 1024], BF16)
            for cc in range(NCC):
                nc.sync.dma_start(out=wqkv_sb[:, cc, :], in_=wqkv[bass.ts(cc, 128), :])
            wp_sb = big.tile([128, 4, 2048], BF16)
            for dc in range(4):
                nc.sync.dma_start(out=wp_sb[:, dc, :], in_=wp[bass.ts(dc, 128), :])
            ve_sb = big.tile([128, NT, 256], BF16)
            for i in range(NT):
                nc.sync.dma_start(out=ve_sb[:, i, :], in_=ve2[bass.ts(i, 128), :])
            cos_sb = big.tile([128, NT, 64], BF16)
            nc.sync.dma_start(out=cos_sb, in_=cosb.rearrange("(i p) d -> p i d", p=128))  # small
            sin_sb = big.tile([128, NT, 64], BF16)
            nc.sync.dma_start(out=sin_sb, in_=sinb.rearrange("(i p) d -> p i d", p=128))
            mdiag_sb = big.tile([128, 128], BF16)
            nc.sync.dma_start(out=mdiag_sb, in_=mdiag[:, :])
            mfar_sb = big.tile([128, 128], BF16)
            nc.sync.dma_start(out=mfar_sb, in_=mfar[:, :])
            wveg_sb = big.tile([32, 2], BF16)
            nc.sync.dma_start(out=wveg_sb, in_=wveg[:, :])
            wag_sb = big.tile([12, 4], BF16)
            nc.sync.dma_start(out=wag_sb, in_=wag[:, :])

            ident = big.tile([128, 128], BF16)
            make_identity(nc, ident)
            eps_ap = big.tile([128, 1], FP32)
            nc.vector.memset(eps_ap, EPS)
            eps128_ap = big.tile([128, 1], FP32)
            nc.vector.memset(eps128_ap, 128.0 * EPS)

            # ---- persistent intermediates ----
            qT_sb = big.tile([128, 4, 2048], BF16)    # [d, h, t] normalized q
            kT_sb = big.tile([128, 2, 2048], BF16)    # [d, hk, t] normalized k
            v_sb = big.tile([128, NT, 2, 132], BF16)  # [t, i, hk, dv(+ones)]
            nc.vector.memset(v_sb[:, :, :, 128:129], 1.0)
            ag_sb = big.tile([128, NT, 4], FP32)      # attn gate per (t, h)
            k_raw = big.tile([128, NT, 2, 128], BF16)  # natural k pre-shift
            k_shift = big.tile([128, NT, 2, 64], BF16)  # shifted upper halves
            yT_sb = big.tile([128, 4, 2048], BF16)    # [dv, h, t]

            with (
                tc.tile_pool(name="pp", bufs=2, space="PSUM") as pp,
                tc.tile_pool(name="pg", bufs=1, space="PSUM") as pg,
                tc.tile_pool(name="ptr", bufs=1, space="PSUM") as ptr,
            ):
                # ---- gates, projections, rope q, rstd, transpose q ----
                for i in range(NT):
                    ts = bass.ts(i, 128)
                    # gates
                    zv_ps = pg.tile([128, 4], FP32, tag="g")
                    nc.tensor.matmul(zv_ps[:, 0:2], xT_sb[0:32, 0, ts], wveg_sb, start=True, stop=True)
                    za_ps = pg.tile([128, 4], FP32, tag="g")
                    nc.tensor.matmul(za_ps, xT_sb[0:12, 0, ts], wag_sb[0:12, :], start=True, stop=True)
                    # sigmoid = recip(1 + exp(-z))
                    gv = small.tile([128, 2], FP32)
                    nc.scalar.activation(gv, zv_ps[:, 0:2], AF.Exp, scale=-1.0)
                    nc.vector.tensor_scalar_add(gv, gv, 1.0)
                    nc.vector.reciprocal(gv, gv)
                    ga = small.tile([128, 4], FP32)
                    nc.scalar.activation(ga, za_ps, AF.Exp, scale=-1.0)
                    nc.vector.tensor_scalar_add(ga, ga, 1.0)
                    nc.vector.reciprocal(ag_sb[:, i, :], ga)

                    # projections for this t-tile
                    q_ps = pp.tile([128, 512], FP32, tag="qps")
                    kv_ps = pp.tile([128, 512], FP32, tag="kvps")
                    for cc in range(NCC):
                        lhsT = xT_sb[:, cc, ts]
                        st = cc == 0
                        sp = cc == NCC - 1
                        nc.tensor.matmul(q_ps, lhsT, wqkv_sb[:, cc, 0:512], start=st, stop=sp)
                        nc.tensor.matmul(kv_ps, lhsT, wqkv_sb[:, cc, 512:1024], start=st, stop=sp)
                    k_ps = kv_ps[:, 0:256]
                    v_ps = kv_ps[:, 256:512]
                    # v with ve gating: v_sb = (ve2 * gv) + v_ps   (2*sigma folded into ve2)
                    for hk in range(2):
                        nc.vector.scalar_tensor_tensor(
                            out=v_sb[:, i, hk, 0:128],
                            in0=ve_sb[:, i, bass.ts(hk, 128)],
                            scalar=gv[:, hk : hk + 1],
                            in1=v_ps[:, bass.ts(hk, 128)],
                            op0=ALU.mult,
                            op1=ALU.add,
                        )
                    # k natural bf16 (pre-shift)
                    nc.vector.tensor_copy(k_raw[:, i, :, :], k_ps.rearrange("p (h d) -> p h d", h=2))
                    # evacuate q psum to sbuf (bf16), rope from sbuf
                    q_nat = work.tile([128, 4, 128], BF16, tag="qnat")
                    nc.vector.tensor_copy(q_nat, q_ps.rearrange("p (h d) -> p h d", h=4))
                    qr = work.tile([128, 4, 128], BF16, tag="qr")
                    q_v = q_nat
                    cb = _bcast_mid(cos_sb[:, i, :], 4)
                    sb = _bcast_mid(sin_sb[:, i, :], 4)
                    t1 = work.tile([128, 4, 64], BF16, tag="tt1")
                    nc.vector.tensor_tensor(t1, q_v[:, :, 0:64], cb, op=ALU.mult)
                    t2 = work.tile([128, 4, 64], BF16, tag="tt2")
                    nc.vector.tensor_tensor(t2, q_v[:, :, 64:128], sb, op=ALU.mult)
                    nc.vector.tensor_tensor(qr[:, :, 0:64], t1, t2, op=ALU.add)
                    nc.vector.tensor_tensor(t1, q_v[:, :, 64:128], cb, op=ALU.mult)
                    nc.vector.tensor_tensor(t2, q_v[:, :, 0:64], sb, op=ALU.mult)
                    nc.vector.tensor_tensor(qr[:, :, 64:128], t1, t2, op=ALU.subtract)
                    # rstd_q = (ssq + 128*eps)^-0.5  [1/sqrt(128) folded in]
                    qsq = work.tile([128, 4, 128], FP32, tag="sq")
                    nc.vector.tensor_tensor(qsq, qr, qr, op=ALU.mult)
                    ssq = small.tile([128, 4], FP32, tag="ssq")
                    nc.vector.tensor_reduce(ssq, qsq, axis=AX.X, op=ALU.add)
                    lnq = small.tile([128, 4], FP32, tag="lnq")
                    nc.scalar.activation(lnq, ssq, AF.Ln, bias=eps128_ap)
                    rstd = small.tile([128, 4], FP32, tag="rstd")
                    nc.scalar.activation(rstd, lnq, AF.Exp, scale=-0.5)
                    for h in range(4):
                        nc.vector.tensor_scalar_mul(qr[:, h, :], qr[:, h, :], rstd[:, h : h + 1])
                    # transpose q -> qT
                    for h in range(4):
                        tp_ps = ptr.tile([128, 128], BF16, tag="tps")
                        nc.tensor.transpose(tp_ps, qr[:, h, :], ident)
                        nc.vector.tensor_copy(qT_sb[:, h, ts], tp_ps)

                # key shift: upper halves move one step along t
                for i in range(NT):
                    nc.sync.dma_start(out=k_shift[1:128, i, :, :], in_=k_raw[0:127, i, :, 64:128])
                    if i == 0:
                        nc.sync.dma_start(out=k_shift[0:1, 0, :, :], in_=k_raw[0:1, 0, :, 64:128])
                    else:
                        nc.sync.dma_start(out=k_shift[0:1, i, :, :], in_=k_raw[127:128, i - 1, :, 64:128])

                # rope+rmsnorm+transpose for k
                for i in range(NT):
                    ts = bass.ts(i, 128)
                    kr = work.tile([128, 2, 128], BF16, tag="kr")
                    k1 = k_raw[:, i, :, 0:64]
                    k2 = k_shift[:, i, :, :]
                    cb = _bcast_mid(cos_sb[:, i, :], 2)
                    sb = _bcast_mid(sin_sb[:, i, :], 2)
                    t1 = work.tile([128, 2, 64], BF16, tag="tt1")
                    nc.vector.tensor_tensor(t1, k1, cb, op=ALU.mult)
                    t2 = work.tile([128, 2, 64], BF16, tag="tt2")
                    nc.vector.tensor_tensor(t2, k2, sb, op=ALU.mult)
                    nc.vector.tensor_tensor(kr[:, :, 0:64], t1, t2, op=ALU.add)
                    nc.vector.tensor_tensor(t1, k2, cb, op=ALU.mult)
                    nc.vector.tensor_tensor(t2, k1, sb, op=ALU.mult)
                    nc.vector.tensor_tensor(kr[:, :, 64:128], t1, t2, op=ALU.subtract)
                    ksq = work.tile([128, 2, 128], FP32, tag="sq")
                    nc.vector.tensor_tensor(ksq, kr, kr, op=ALU.mult)
                    ssk = small.tile([128, 2], FP32, tag="ssk")
                    nc.vector.tensor_reduce(ssk, ksq, axis=AX.X, op=ALU.add)
                    lnk = small.tile([128, 2], FP32, tag="lnk")
                    nc.scalar.activation(lnk, ssk, AF.Ln, bias=eps_ap, scale=1.0 / 128.0)
                    rstdk = small.tile([128, 2], FP32, tag="rstdk")
                    nc.scalar.activation(rstdk, lnk, AF.Exp, scale=-0.5)
                    for h in range(2):
                        nc.vector.tensor_scalar_mul(kr[:, h, :], kr[:, h, :], rstdk[:, h : h + 1])
                    for h in range(2):
                        tp_ps = ptr.tile([128, 128], BF16, tag="tps")
                        nc.tensor.transpose(tp_ps, kr[:, h, :], ident)
                        nc.vector.tensor_copy(kT_sb[:, h, ts], tp_ps)

            # ---- attention + wproj ----
            with (
                tc.tile_pool(name="pst", bufs=2, space="PSUM") as pst,
                tc.tile_pool(name="py", bufs=2, space="PSUM") as py,
            ):
                for i in range(NT):
                    for h in range(4):
                        hk = h // 2
                        js = list(range(max(0, i - 8), i + 1))
                        st_ps = pst.tile([128, 9, 128], FP32, tag="st")
                        for idx, j in enumerate(js):
                            nc.tensor.matmul(
                                st_ps[:, idx, :],
                                kT_sb[:, hk, bass.ts(j, 128)],
                                qT_sb[:, h, bass.ts(i, 128)],
                                start=True, stop=True,
                            )
                        ex = work.tile([128, 9, 128], BF16, tag="ex")
                        nc.scalar.activation(ex[:, 0 : len(js), :], st_ps[:, 0 : len(js), :], AF.Exp)
                        # masks (multiplicative, after exp)
                        nc.vector.tensor_tensor(ex[:, len(js) - 1, :], ex[:, len(js) - 1, :], mdiag_sb, op=ALU.mult)
                        if i >= 8:
                            nc.vector.tensor_tensor(ex[:, 0, :], ex[:, 0, :], mfar_sb, op=ALU.mult)
                        y_ps = py.tile([128, 512], FP32, tag="yo")
                        for idx, j in enumerate(js):
                            nc.tensor.matmul(
                                y_ps[:, 0:129],
                                ex[:, idx, :],
                                v_sb[:, j, hk, 0:129],
                                start=(idx == 0), stop=(idx == len(js) - 1),
                            )
                        # factor = ag / rowsum
                        rs = small.tile([128, 1], FP32, tag="rs")
                        nc.vector.reciprocal(rs, y_ps[:, 128:129])
                        fac = small.tile([128, 1], FP32, tag="fac")
                        nc.vector.tensor_tensor(fac, rs, ag_sb[:, i, h : h + 1], op=ALU.mult)
                        yn = work.tile([128, 128], BF16, tag="yn")
                        nc.vector.tensor_scalar_mul(yn, y_ps[:, 0:128], fac)
                        tp_ps = py.tile([128, 512], BF16, tag="yo")
                        nc.tensor.transpose(tp_ps[:, 0:128], yn, ident)
                        nc.vector.tensor_copy(yT_sb[:, h, bass.ts(i, 128)], tp_ps[:, 0:128])

                # ---- wproj ----
                for i in range(NT):
                    ts = bass.ts(i, 128)
                    for c in range(4):
                        o_ps = py.tile([128, 512], FP32, tag="yo")
                        for dc in range(4):
                            nc.tensor.matmul(
                                o_ps,
                                yT_sb[:, dc, ts],
                                wp_sb[:, dc, bass.ts(c, 512)],
                                start=(dc == 0), stop=(dc == 3),
                            )
                        o_sb = work.tile([128, 512], FP32, tag="osb")
                        nc.scalar.copy(o_sb, o_ps)
                        nc.sync.dma_start(out=out[ts, bass.ts(c, 512)], in_=o_sb)
    nc.compile()
    return nc


def _get_nc():
    if "nc" not in _CACHE:
        _CACHE["nc"] = _build_nc()
    return _CACHE["nc"]


def kernel(**inputs):
    x = np.asarray(inputs["x"], np.float32)
    ve = np.asarray(inputs["ve"], np.float32)
    cos = np.asarray(inputs["cos"], np.float32).reshape(T, 64)
    sin = np.asarray(inputs["sin"], np.float32).reshape(T, 64)
    wq = np.asarray(inputs["wq"], np.float32)
    wk = np.asarray(inputs["wk"], np.float32)
    wv = np.asarray(inputs["wv"], np.float32)
    wproj = np.asarray(inputs["wproj"], np.float32)
    wveg = np.asarray(inputs["w_ve_gate"], np.float32)
    wag = np.asarray(inputs["w_attn_gate"], np.float32)
    proj_scalar = np.asarray(inputs["proj_scalar"], np.float32)

    ii, jj = np.meshgrid(np.arange(128), np.arange(128), indexing="ij")
    mdiag = (jj >= ii).astype(bf16)   # [k, q]: allowed q >= k
    mfar = (jj <= ii).astype(bf16)    # [k, q]: allowed q <= k
    cosb = cos.astype(bf16)
    sinb = sin.astype(bf16)

    in_maps = []
    for core in range(8):
        b, tp = core // 4, core % 4
        in_maps.append({
            "xT": np.ascontiguousarray(x[b].T).astype(bf16),
            "ve2": (2.0 * ve[b][:, tp * 256 : (tp + 1) * 256]).astype(bf16),
            "wqkv": np.ascontiguousarray(np.concatenate([
                wq[:, tp * 512 : (tp + 1) * 512],
                wk[:, tp * 256 : (tp + 1) * 256],
                wv[:, tp * 256 : (tp + 1) * 256]], axis=1)).astype(bf16),
            "wp": np.ascontiguousarray(wproj[tp * 512 : (tp + 1) * 512, :]).astype(bf16),
            "wveg": np.ascontiguousarray(wveg[:, 2 * tp : 2 * tp + 2]).astype(bf16),
            "wag": np.ascontiguousarray(wag[:, 4 * tp : 4 * tp + 4]).astype(bf16),
            "cosb": cosb, "sinb": sinb, "mdiag": mdiag, "mfar": mfar,
        })

    import os
    trace = bool(os.environ.get("BASS_KERNEL_TRACE"))
    res = run_bass_kernel_spmd(_get_nc(), in_maps, core_ids=list(range(8)), trace=trace)
    if trace:
        _CACHE["last_res"] = res
    out = np.zeros((2, T, 2048), np.float32)
    for core in range(8):
        b = core // 4
        out[b] += res.results[core]["out"]
    out *= (1.0 + proj_scalar[0])
    return out

